# revision 22
# baseline (speedup 1.0000x reference)
"""Trainium2 Bass kernel for CLIPAttention with 2D interleaved RoPE.

Problem: B=16, T=1024, E=1024, H=16, DH=64, f32 in/out.
Sharding: data-parallel over batch across 8 NeuronCores (2 batches/core).

v3: bf16 matmul operands (f32 PSUM accumulation, f32 softmax exp on ACT),
restructured for overlap:
  - dedicated PSUM pools per stage (proj / scores / attn-out) so the Tile
    scheduler can run stage N+1 matmuls during stage N's ACT/DVE work
  - attention loops tq-outer, heads of a pair interleaved (their scores
    matmuls use disjoint 64-row groups, so weight loads overlap matmuls)
  - weights Wv/Wo loaded once for both batches; x/trig DMA'd per e-chunk and
    double-buffered across batches
  - Q/K PSUM->SBUF copies (with bias) on DVE, exp on ACT

Per-core algorithm per batch:
  host prep:  xT [E,T] bf16; W^T [e,o] bf16; q/k output dims permuted per
              head so RoPE pair partners sit 32 partitions apart; trig
              tables ccat/scat [128,T] bf16 with signs folded; P (block-swap)
              bf16; biases f32.
  device:     V = x@Wv^T +bv (natural [t,o]) -> VV tiles [tk,65] bf16 with
              ones col per head; per head-pair hp:
                QT,KT = (W^T slab).T @ xT (+bias via DVE, out bf16)
                rope: rot = q*ccat + (P@q)*scat  (swap via PE matmul)
                per tq-half, heads interleaved:
                  scoresT[tk,tq] = KT.T@QT; expT = exp(scale*scoresT) bf16
                  outT[0:65] = [V|1].T @ expT (accum over tk chunks)
                  denom = row 64; attnT = outT[0:64] * bcast(1/denom) -> bf16
              y = attnT.T-chunks @ Wo^T + bo  (f32, natural [t,o]) -> DRAM
Softmax skips max-subtraction: |scores*scale| <~ 8 for these inputs, exp is
exact in fp32 there, and softmax is shift-invariant.
"""
import numpy as np

B, T, E, H = 16, 1024, 1024, 16
DH = E // H            # 64
THETA = 10000.0
N_CORES = 8
BPC = B // N_CORES     # 2 batches per core
HP = H // 2            # 8 head pairs
EC = E // 128          # 8 e-chunks
HALF, QUARTER = DH // 2, DH // 4   # 32, 16
SCALE = float(DH) ** -0.5

_compiled_nc = None


def _build_nc(reps=1, hoist_dma=False):
    import concourse.bacc as bacc
    import concourse.tile as tile
    from concourse import mybir
    from contextlib import ExitStack, nullcontext

    f32 = mybir.dt.float32
    bf16 = mybir.dt.bfloat16
    FT = mybir.ActivationFunctionType

    nc = bacc.Bacc("TRN2", target_bir_lowering=False)

    xt_d = nc.dram_tensor("xt", [BPC, E, T], bf16, kind="ExternalInput")
    wqt_d = nc.dram_tensor("wqt", [E, E], bf16, kind="ExternalInput")
    wkt_d = nc.dram_tensor("wkt", [E, E], bf16, kind="ExternalInput")
    wvt_d = nc.dram_tensor("wvt", [E, E], bf16, kind="ExternalInput")
    wot_d = nc.dram_tensor("wot", [E, E], bf16, kind="ExternalInput")
    pmat_d = nc.dram_tensor("pmat", [128, 128], bf16, kind="ExternalInput")
    ccat_d = nc.dram_tensor("ccat", [BPC, 128, T], bf16, kind="ExternalInput")
    scat_d = nc.dram_tensor("scat", [BPC, 128, T], bf16, kind="ExternalInput")
    bqk_d = nc.dram_tensor("bqk", [128, 2 * HP], f32, kind="ExternalInput")
    bv_d = nc.dram_tensor("bv", [128, E], f32, kind="ExternalInput")
    bo_d = nc.dram_tensor("bo", [128, E], f32, kind="ExternalInput")
    y_d = nc.dram_tensor("y", [BPC, T, E], f32, kind="ExternalOutput")

    def wslab_ap(w, hp):
        return w.ap().rearrange("(c p) o -> p c o", p=128)[:, :, hp * 128:(hp + 1) * 128]

    with tile.TileContext(nc) as tc, ExitStack() as ctx:
        pre = ctx.enter_context(tc.tile_pool(name="pre", bufs=1)) \
            if hoist_dma else None
        hoisted = {}
        if hoist_dma:
            # stage the big inputs once, outside the timing loop
            for b in range(BPC):
                hx = pre.tile([128, EC, T], bf16, tag=f"hx{b}")
                nc.sync.dma_start(
                    hx[:], xt_d.ap()[b].rearrange("(c p) t -> p c t", p=128))
                hc = pre.tile([128, T], bf16, tag=f"hc{b}")
                nc.sync.dma_start(hc[:], ccat_d.ap()[b])
                hs = pre.tile([128, T], bf16, tag=f"hs{b}")
                nc.sync.dma_start(hs[:], scat_d.ap()[b])
                hoisted[b] = (hx, hc, hs)
            hwv = pre.tile([128, EC, E], bf16, tag="hwv")
            nc.sync.dma_start(
                hwv[:], wvt_d.ap().rearrange("(c p) o -> p c o", p=128))
            hwo = pre.tile([128, EC, E], bf16, tag="hwo")
            nc.sync.dma_start(
                hwo[:], wot_d.ap().rearrange("(c p) o -> p c o", p=128))
        ctx.enter_context(tc.For_i(0, reps, 1) if reps > 1 else nullcontext())
        const = ctx.enter_context(tc.tile_pool(name="const", bufs=1))
        if not hoist_dma:
            wvwo = ctx.enter_context(tc.tile_pool(name="wvwo", bufs=2))
            trig = ctx.enter_context(tc.tile_pool(name="trig", bufs=2))
            xtp = ctx.enter_context(tc.tile_pool(name="xtp", bufs=2))
        wslab = ctx.enter_context(tc.tile_pool(name="wslab", bufs=4))
        qkp = ctx.enter_context(tc.tile_pool(name="qkp", bufs=3))
        rotp = ctx.enter_context(tc.tile_pool(name="rotp", bufs=3))
        tmpp = ctx.enter_context(tc.tile_pool(name="tmpp", bufs=4))
        vvp = ctx.enter_context(tc.tile_pool(name="vvp", bufs=2))
        expp = ctx.enter_context(tc.tile_pool(name="expp", bufs=6))
        attnp = ctx.enter_context(tc.tile_pool(name="attnp", bufs=1))
        smallp = ctx.enter_context(tc.tile_pool(name="smallp", bufs=4))
        yp = ctx.enter_context(tc.tile_pool(name="yp", bufs=2))
        psP = ctx.enter_context(tc.tile_pool(name="psP", bufs=2, space="PSUM"))
        psS = ctx.enter_context(tc.tile_pool(name="psS", bufs=2, space="PSUM"))
        psO = ctx.enter_context(tc.tile_pool(name="psO", bufs=2, space="PSUM"))

        pm = const.tile([128, 128], bf16, tag="pm")
        nc.sync.dma_start(pm[:], pmat_d.ap())
        bqk_sb = const.tile([128, 2 * HP], f32, tag="bqk")
        nc.sync.dma_start(bqk_sb[:], bqk_d.ap())
        bv_sb = const.tile([128, E], f32, tag="bv")
        nc.sync.dma_start(bv_sb[:], bv_d.ap())
        bo_sb = const.tile([128, E], f32, tag="bo")
        nc.sync.dma_start(bo_sb[:], bo_d.ap())

        # Wv / Wo: shared by both batches, load once (chunked per e-chunk).
        # Wo's DMA is queued after batch 0's inputs — it is needed ~300us in.
        if hoist_dma:
            wv_sb, wo_sb = hwv, hwo
        else:
            wv_sb = wvwo.tile([128, EC, E], bf16, tag="wvwo", name="wv")
            wo_sb = wvwo.tile([128, EC, E], bf16, tag="wvwo", name="wo")

        for b in range(BPC):
            if hoist_dma:
                xts, cc, sc = hoisted[b]
            else:
                xts = xtp.tile([128, EC, T], bf16, tag="xts")
                for ec in range(EC):
                    nc.sync.dma_start(
                        xts[:, ec],
                        xt_d.ap()[b].rearrange("(c p) t -> p c t", p=128)[:, ec])
                    if b == 0:
                        nc.sync.dma_start(
                            wv_sb[:, ec],
                            wvt_d.ap().rearrange("(c p) o -> p c o", p=128)[:, ec])
                cc = trig.tile([128, T], bf16, tag="cc")
                nc.sync.dma_start(cc[:], ccat_d.ap()[b])
                sc = trig.tile([128, T], bf16, tag="sc")
                nc.sync.dma_start(sc[:], scat_d.ap()[b])
            if b == 0 and not hoist_dma:
                for ec in range(EC):
                    nc.sync.dma_start(
                        wo_sb[:, ec],
                        wot_d.ap().rearrange("(c p) o -> p c o", p=128)[:, ec])

            # ---- V phase: V natural [t, o] for all heads -> VV tiles ----
            vvt = vvp.tile([128, EC, H, DH + 1], bf16, tag="vv")
            nc.gpsimd.memset(vvt[:, :, :, DH:DH + 1], 1.0)
            for tcn in range(EC):
                for oh in range(2):
                    vps = psP.tile([128, 512], f32, tag="psP",
                                   name=f"vps{tcn}_{oh}")
                    for ec in range(EC):
                        nc.tensor.matmul(
                            vps[:],
                            xts[:, ec, tcn * 128:(tcn + 1) * 128],
                            wv_sb[:, ec, oh * 512:(oh + 1) * 512],
                            start=(ec == 0), stop=(ec == EC - 1))
                    nc.vector.tensor_add(
                        vvt[:, tcn, oh * 8:(oh + 1) * 8, 0:DH],
                        vps[:].rearrange("p (h d) -> p h d", d=DH),
                        bv_sb[:, oh * 512:(oh + 1) * 512]
                        .rearrange("p (h d) -> p h d", d=DH))

            attn_sb = attnp.tile([128, EC, T], bf16, tag="attn")

            # ---- per head-pair: Q/K projection + rope + attention ----
            for hp in range(HP):
                wq_sb = wslab.tile([128, EC, 128], bf16, tag="wq")
                nc.sync.dma_start(wq_sb[:], wslab_ap(wqt_d, hp))
                wk_sb = wslab.tile([128, EC, 128], bf16, tag="wk")
                nc.sync.dma_start(wk_sb[:], wslab_ap(wkt_d, hp))

                rots = []
                for ti, wsb in enumerate((wq_sb, wk_sb)):
                    sb = qkp.tile([128, T], bf16, tag="qksb")
                    rot = rotp.tile([128, T], bf16, tag=("rotq", "rotk")[ti])
                    t1 = tmpp.tile([128, T], bf16, tag="t1")
                    for tq in range(2):
                        ts = slice(tq * 512, (tq + 1) * 512)
                        pps = psP.tile([128, 512], f32, tag="psP",
                                       name=f"pps{ti}_{tq}")
                        for ec in range(EC):
                            nc.tensor.matmul(
                                pps[:], wsb[:, ec, :], xts[:, ec, ts],
                                start=(ec == 0), stop=(ec == EC - 1))
                        nc.vector.tensor_scalar_add(
                            sb[:, ts], pps[:],
                            bqk_sb[:, ti * HP + hp:ti * HP + hp + 1])
                        sps = psP.tile([128, 512], f32, tag="psP",
                                       name=f"sps{ti}_{tq}")
                        nc.tensor.matmul(sps[:], pm[:], sb[:, ts],
                                         start=True, stop=True)
                        t2 = tmpp.tile([128, 512], bf16, tag="t2")
                        nc.vector.tensor_mul(t2[:], sps[:], sc[:, ts])
                        nc.vector.tensor_mul(t1[:, ts], sb[:, ts], cc[:, ts])
                        nc.vector.tensor_add(rot[:, ts], t1[:, ts], t2[:])
                    rots.append(rot)
                qrot, krot = rots

                for tq in range(2):
                    o_ps = [psO.tile([DH + 1, 512], f32, tag="psO",
                                     name=f"ops{hh}") for hh in range(2)]
                    # software pipeline: scores/exp of chunk k+1 are issued
                    # before the AV matmuls of chunk k, so the in-order PE
                    # never waits on ACT's exp.
                    prev = None
                    for tkc in range(EC):
                        # both heads' scores into one 2-bank tile -> one exp
                        scps = psS.tile([128, T], f32, tag="psS", name="sc2")
                        for hh in range(2):
                            qh = qrot[hh * 64:(hh + 1) * 64, :]
                            kh = krot[hh * 64:(hh + 1) * 64, :]
                            nc.tensor.matmul(
                                scps[:, hh * 512:(hh + 1) * 512],
                                kh[:, tkc * 128:(tkc + 1) * 128],
                                qh[:, tq * 512:(tq + 1) * 512],
                                start=True, stop=True)
                        ext = expp.tile([128, T], bf16, tag="ext")
                        nc.scalar.activation(ext[:], scps[:], FT.Exp,
                                             scale=SCALE)
                        if prev is not None:
                            for hh in range(2):
                                nc.tensor.matmul(
                                    o_ps[hh][:],
                                    vvt[:, tkc - 1, 2 * hp + hh, :],
                                    prev[:, hh * 512:(hh + 1) * 512],
                                    start=(tkc - 1 == 0), stop=False)
                        prev = ext
                    for hh in range(2):
                        nc.tensor.matmul(
                            o_ps[hh][:],
                            vvt[:, EC - 1, 2 * hp + hh, :],
                            prev[:, hh * 512:(hh + 1) * 512],
                            start=False, stop=True)
                    for hh in range(2):
                        rc = smallp.tile([1, 512], f32, tag="rc")
                        nc.vector.reciprocal(rc[:], o_ps[hh][DH:DH + 1, :])
                        rcb = smallp.tile([64, 512], f32, tag="rcb")
                        nc.gpsimd.partition_broadcast(rcb[:], rc[:])
                        nc.vector.tensor_mul(
                            attn_sb[hh * 64:(hh + 1) * 64, hp,
                                    tq * 512:(tq + 1) * 512],
                            o_ps[hh][0:DH, :], rcb[:])

            # ---- out-proj: y[t, o] = attnT.T-chunks @ WoT + bo ----
            for tcn in range(EC):
                for oh in range(2):
                    yps = psP.tile([128, 512], f32, tag="psP",
                                   name=f"yps{oh}")
                    for ec in range(EC):
                        nc.tensor.matmul(
                            yps[:],
                            attn_sb[:, ec, tcn * 128:(tcn + 1) * 128],
                            wo_sb[:, ec, oh * 512:(oh + 1) * 512],
                            start=(ec == 0), stop=(ec == EC - 1))
                    ysb = yp.tile([128, 512], f32, tag="y")
                    nc.vector.tensor_add(ysb[:], yps[:],
                                         bo_sb[:, oh * 512:(oh + 1) * 512])
                    nc.sync.dma_start(
                        y_d.ap()[b, tcn * 128:(tcn + 1) * 128,
                                 oh * 512:(oh + 1) * 512], ysb[:])

    nc.compile()
    return nc


def _host_prep(inputs):
    import ml_dtypes
    bf16 = ml_dtypes.bfloat16

    x = np.asarray(inputs["hidden_states"], dtype=np.float32)
    rope_pos = np.asarray(inputs["rope_pos"])

    # per-head permutation: [h-half evens, w-half evens, h-half odds, w-half odds]
    p64 = np.concatenate([
        np.arange(0, HALF, 2), np.arange(HALF, DH, 2),
        np.arange(1, HALF, 2), np.arange(HALF + 1, DH, 2)])
    perm = np.concatenate([h * DH + p64 for h in range(H)])

    wqt = np.ascontiguousarray(np.asarray(inputs["Wq"], np.float32).T[:, perm]).astype(bf16)
    wkt = np.ascontiguousarray(np.asarray(inputs["Wk"], np.float32).T[:, perm]).astype(bf16)
    wvt = np.ascontiguousarray(np.asarray(inputs["Wv"], np.float32).T).astype(bf16)
    wot = np.ascontiguousarray(np.asarray(inputs["Wo"], np.float32).T).astype(bf16)
    bq_p = np.asarray(inputs["bq"], np.float32)[perm]
    bk_p = np.asarray(inputs["bk"], np.float32)[perm]
    bv = np.asarray(inputs["bv"], np.float32)
    bo = np.asarray(inputs["bo"], np.float32)

    # bqk [128, 2*HP]: col ti*HP+hp = bias for slab hp of (q if ti==0 else k)
    bqk = np.empty((128, 2 * HP), np.float32)
    for hp in range(HP):
        bqk[:, hp] = bq_p[hp * 128:(hp + 1) * 128]
        bqk[:, HP + hp] = bk_p[hp * 128:(hp + 1) * 128]
    bv_rep = np.ascontiguousarray(np.broadcast_to(bv, (128, E)))
    bo_rep = np.ascontiguousarray(np.broadcast_to(bo, (128, E)))

    # trig tables, f32 pipeline mirroring the reference, then bf16
    idx = np.arange(QUARTER, dtype=np.float32)
    inv = (np.float32(THETA) ** (np.float32(-2.0) * idx / np.float32(QUARTER))
           ).astype(np.float32)
    pos = rope_pos.astype(np.float32)                    # [B, T, 2]
    ang_h = pos[:, :, 0:1] * inv                         # [B, T, 16]
    ang_w = pos[:, :, 1:2] * inv
    ch, cw = np.cos(ang_h), np.cos(ang_w)
    sh, sw = np.sin(ang_h), np.sin(ang_w)
    cos64 = np.concatenate([ch, cw, ch, cw], axis=2)     # [B, T, 64]
    sin64 = np.concatenate([-sh, -sw, sh, sw], axis=2)
    ccat = np.ascontiguousarray(np.transpose(cos64, (0, 2, 1)))  # [B, 64, T]
    scat = np.ascontiguousarray(np.transpose(sin64, (0, 2, 1)))
    ccat = np.ascontiguousarray(np.concatenate([ccat, ccat], axis=1)).astype(bf16)
    scat = np.ascontiguousarray(np.concatenate([scat, scat], axis=1)).astype(bf16)

    pmat = np.zeros((128, 128), np.float32)
    for base in (0, 64):
        pmat[base:base + 32, base + 32:base + 64] = np.eye(32)
        pmat[base + 32:base + 64, base:base + 32] = np.eye(32)
    pmat = pmat.astype(bf16)

    xt_all = np.transpose(x, (0, 2, 1)).astype(bf16)     # [B, E, T]

    in_maps = []
    for c in range(N_CORES):
        bs = slice(c * BPC, (c + 1) * BPC)
        in_maps.append({
            "xt": np.ascontiguousarray(xt_all[bs]),
            "wqt": wqt, "wkt": wkt, "wvt": wvt, "wot": wot,
            "pmat": pmat,
            "ccat": np.ascontiguousarray(ccat[bs]),
            "scat": np.ascontiguousarray(scat[bs]),
            "bqk": bqk, "bv": bv_rep, "bo": bo_rep,
        })
    return in_maps


PROFILE = False
LAST_RESULT = None


def kernel(**inputs):
    global _compiled_nc, LAST_RESULT
    from concourse.bass_utils import run_bass_kernel_spmd

    if _compiled_nc is None:
        _compiled_nc = _build_nc()
    in_maps = _host_prep(inputs)
    res = run_bass_kernel_spmd(_compiled_nc, in_maps, list(range(N_CORES)),
                               trace=PROFILE)
    LAST_RESULT = res
    out = np.concatenate([res.results[c]["y"] for c in range(N_CORES)], axis=0)
    return out.astype(np.float32)


# revision 29
# speedup vs baseline: 1.0945x; 1.0945x over previous
"""Trainium2 Bass kernel for CLIPAttention with 2D interleaved RoPE.

Problem: B=16, T=1024, E=1024, H=16, DH=64, f32 in/out.
Sharding: data-parallel over batch across 8 NeuronCores (2 batches/core).

v3: bf16 matmul operands (f32 PSUM accumulation, f32 softmax exp on ACT),
restructured for overlap:
  - dedicated PSUM pools per stage (proj / scores / attn-out) so the Tile
    scheduler can run stage N+1 matmuls during stage N's ACT/DVE work
  - attention loops tq-outer, heads of a pair interleaved (their scores
    matmuls use disjoint 64-row groups, so weight loads overlap matmuls)
  - weights Wv/Wo loaded once for both batches; x/trig DMA'd per e-chunk and
    double-buffered across batches
  - Q/K PSUM->SBUF copies (with bias) on DVE, exp on ACT

Per-core algorithm per batch:
  host prep:  xT [E,T] bf16; W^T [e,o] bf16; q/k output dims permuted per
              head so RoPE pair partners sit 32 partitions apart; trig
              tables ccat/scat [128,T] bf16 with signs folded; P (block-swap)
              bf16; biases f32.
  device:     V = x@Wv^T +bv (natural [t,o]) -> VV tiles [tk,65] bf16 with
              ones col per head; per head-pair hp:
                QT,KT = (W^T slab).T @ xT (+bias via DVE, out bf16)
                rope: rot = q*ccat + (P@q)*scat  (swap via PE matmul)
                per tq-half, heads interleaved:
                  scoresT[tk,tq] = KT.T@QT; expT = exp(scale*scoresT) bf16
                  outT[0:65] = [V|1].T @ expT (accum over tk chunks)
                  denom = row 64; attnT = outT[0:64] * bcast(1/denom) -> bf16
              y = attnT.T-chunks @ Wo^T + bo  (f32, natural [t,o]) -> DRAM
Softmax skips max-subtraction: |scores*scale| <~ 8 for these inputs, exp is
exact in fp32 there, and softmax is shift-invariant.
"""
import numpy as np

B, T, E, H = 16, 1024, 1024, 16
DH = E // H            # 64
THETA = 10000.0
N_CORES = 8
BPC = B // N_CORES     # 2 batches per core
HP = H // 2            # 8 head pairs
EC = E // 128          # 8 e-chunks
HALF, QUARTER = DH // 2, DH // 4   # 32, 16
SCALE = float(DH) ** -0.5

_compiled_nc = None


def _build_nc(reps=1, hoist_dma=False):
    import concourse.bacc as bacc
    import concourse.tile as tile
    from concourse import mybir
    from contextlib import ExitStack, nullcontext

    f32 = mybir.dt.float32
    bf16 = mybir.dt.bfloat16
    FT = mybir.ActivationFunctionType

    nc = bacc.Bacc("TRN2", target_bir_lowering=False)

    xt_d = nc.dram_tensor("xt", [BPC, E, T], bf16, kind="ExternalInput")
    wqt_d = nc.dram_tensor("wqt", [E, E], bf16, kind="ExternalInput")
    wkt_d = nc.dram_tensor("wkt", [E, E], bf16, kind="ExternalInput")
    wvt_d = nc.dram_tensor("wvt", [E, E], bf16, kind="ExternalInput")
    wot_d = nc.dram_tensor("wot", [E, E], bf16, kind="ExternalInput")
    pmat_d = nc.dram_tensor("pmat", [128, 128], bf16, kind="ExternalInput")
    ccat_d = nc.dram_tensor("ccat", [BPC, 128, T], bf16, kind="ExternalInput")
    scat_d = nc.dram_tensor("scat", [BPC, 128, T], bf16, kind="ExternalInput")
    bqk_d = nc.dram_tensor("bqk", [128, 2 * HP], f32, kind="ExternalInput")
    bv_d = nc.dram_tensor("bv", [128, E], f32, kind="ExternalInput")
    bo_d = nc.dram_tensor("bo", [128, E], f32, kind="ExternalInput")
    y_d = nc.dram_tensor("y", [BPC, T, E], f32, kind="ExternalOutput")

    def wslab_ap(w, hp):
        return w.ap().rearrange("(c p) o -> p c o", p=128)[:, :, hp * 128:(hp + 1) * 128]

    with tile.TileContext(nc) as tc, ExitStack() as ctx:
        pre = ctx.enter_context(tc.tile_pool(name="pre", bufs=1)) \
            if hoist_dma else None
        hoisted = {}
        if hoist_dma:
            # stage the big inputs once, outside the timing loop
            for b in range(BPC):
                hx = pre.tile([128, EC, T], bf16, tag=f"hx{b}")
                nc.sync.dma_start(
                    hx[:], xt_d.ap()[b].rearrange("(c p) t -> p c t", p=128))
                hc = pre.tile([128, T], bf16, tag=f"hc{b}")
                nc.sync.dma_start(hc[:], ccat_d.ap()[b])
                hs = pre.tile([128, T], bf16, tag=f"hs{b}")
                nc.sync.dma_start(hs[:], scat_d.ap()[b])
                hoisted[b] = (hx, hc, hs)
            hwv = pre.tile([128, EC, E], bf16, tag="hwv")
            nc.sync.dma_start(
                hwv[:], wvt_d.ap().rearrange("(c p) o -> p c o", p=128))
            hwo = pre.tile([128, EC, E], bf16, tag="hwo")
            nc.sync.dma_start(
                hwo[:], wot_d.ap().rearrange("(c p) o -> p c o", p=128))
        ctx.enter_context(tc.For_i(0, reps, 1) if reps > 1 else nullcontext())
        const = ctx.enter_context(tc.tile_pool(name="const", bufs=1))
        if not hoist_dma:
            wvwo = ctx.enter_context(tc.tile_pool(name="wvwo", bufs=2))
            trig = ctx.enter_context(tc.tile_pool(name="trig", bufs=1))
            xtp = ctx.enter_context(tc.tile_pool(name="xtp", bufs=2))
        wslab = ctx.enter_context(tc.tile_pool(name="wslab", bufs=1))
        qkp = ctx.enter_context(tc.tile_pool(name="qkp", bufs=2))
        rotp = ctx.enter_context(tc.tile_pool(name="rotp", bufs=2))
        tmpp = ctx.enter_context(tc.tile_pool(name="tmpp", bufs=3))
        vvp = ctx.enter_context(tc.tile_pool(name="vvp", bufs=2))
        expp = ctx.enter_context(tc.tile_pool(name="expp", bufs=3))
        attnp = ctx.enter_context(tc.tile_pool(name="attnp", bufs=1))
        smallp = ctx.enter_context(tc.tile_pool(name="smallp", bufs=2))
        yp = ctx.enter_context(tc.tile_pool(name="yp", bufs=2))
        psP = ctx.enter_context(tc.tile_pool(name="psP", bufs=2, space="PSUM"))
        psS = ctx.enter_context(tc.tile_pool(name="psS", bufs=1, space="PSUM"))
        psO = ctx.enter_context(tc.tile_pool(name="psO", bufs=2, space="PSUM"))

        pm = const.tile([128, 128], bf16, tag="pm")
        nc.sync.dma_start(pm[:], pmat_d.ap())
        bqk_sb = const.tile([128, 2 * HP], f32, tag="bqk")
        nc.sync.dma_start(bqk_sb[:], bqk_d.ap())
        bv_sb = const.tile([128, E], f32, tag="bv")
        nc.sync.dma_start(bv_sb[:], bv_d.ap())
        bo_sb = const.tile([128, E], f32, tag="bo")
        nc.sync.dma_start(bo_sb[:], bo_d.ap())

        # Wv / Wo: shared by both batches, load once (chunked per e-chunk).
        # Wo's DMA is queued after batch 0's inputs — it is needed ~300us in.
        if hoist_dma:
            wv_sb, wo_sb = hwv, hwo
        else:
            wv_sb = wvwo.tile([128, EC, E], bf16, tag="wvwo", name="wv")
            wo_sb = wvwo.tile([128, EC, E], bf16, tag="wvwo", name="wo")

        for b in range(BPC):
            if hoist_dma:
                xts, cc, sc = hoisted[b]
            else:
                xts = xtp.tile([128, EC, T], bf16, tag="xts")
                for ec in range(EC):
                    nc.sync.dma_start(
                        xts[:, ec],
                        xt_d.ap()[b].rearrange("(c p) t -> p c t", p=128)[:, ec])
                    if b == 0:
                        nc.sync.dma_start(
                            wv_sb[:, ec],
                            wvt_d.ap().rearrange("(c p) o -> p c o", p=128)[:, ec])
                cc = trig.tile([128, T], bf16, tag="cc")
                nc.sync.dma_start(cc[:], ccat_d.ap()[b])
                sc = trig.tile([128, T], bf16, tag="sc")
                nc.sync.dma_start(sc[:], scat_d.ap()[b])
            if b == 0 and not hoist_dma:
                for ec in range(EC):
                    nc.sync.dma_start(
                        wo_sb[:, ec],
                        wot_d.ap().rearrange("(c p) o -> p c o", p=128)[:, ec])

            # ---- V phase: V natural [t, o] for all heads -> VV tiles ----
            vvt = vvp.tile([128, EC, H, DH + 1], bf16, tag="vv")
            nc.gpsimd.memset(vvt[:, :, :, DH:DH + 1], 1.0)
            for tcn in range(EC):
                for oh in range(2):
                    vps = psP.tile([128, 512], f32, tag="psP",
                                   name=f"vps{tcn}_{oh}")
                    for ec in range(EC):
                        nc.tensor.matmul(
                            vps[:],
                            xts[:, ec, tcn * 128:(tcn + 1) * 128],
                            wv_sb[:, ec, oh * 512:(oh + 1) * 512],
                            start=(ec == 0), stop=(ec == EC - 1))
                    nc.vector.tensor_add(
                        vvt[:, tcn, oh * 8:(oh + 1) * 8, 0:DH],
                        vps[:].rearrange("p (h d) -> p h d", d=DH),
                        bv_sb[:, oh * 512:(oh + 1) * 512]
                        .rearrange("p (h d) -> p h d", d=DH))

            attn_sb = attnp.tile([128, EC, T], bf16, tag="attn")

            # Q/K weight slabs for the whole batch: two big DMAs
            wqk_sb = wslab.tile([128, 2, EC, E], bf16, tag="wqk")
            nc.sync.dma_start(
                wqk_sb[:, 0], wqt_d.ap().rearrange("(c p) o -> p c o", p=128))
            nc.sync.dma_start(
                wqk_sb[:, 1], wkt_d.ap().rearrange("(c p) o -> p c o", p=128))

            # ---- per head-pair: Q/K projection + rope + attention ----
            for hp in range(HP):
                rots = []
                for ti in range(2):
                    sb = qkp.tile([128, T], bf16, tag="qksb")
                    rot = rotp.tile([128, T], bf16, tag=("rotq", "rotk")[ti])
                    t1 = tmpp.tile([128, T], bf16, tag="t1")
                    t2 = tmpp.tile([128, T], bf16, tag="t2")
                    for tq in range(2):
                        ts = slice(tq * 512, (tq + 1) * 512)
                        pps = psP.tile([128, 512], f32, tag="psP",
                                       name=f"pps{ti}_{tq}")
                        for ec in range(EC):
                            nc.tensor.matmul(
                                pps[:],
                                wqk_sb[:, ti, ec, hp * 128:(hp + 1) * 128],
                                xts[:, ec, ts],
                                start=(ec == 0), stop=(ec == EC - 1))
                        nc.vector.tensor_scalar_add(
                            sb[:, ts], pps[:],
                            bqk_sb[:, ti * HP + hp:ti * HP + hp + 1])
                        sps = psP.tile([128, 512], f32, tag="psP",
                                       name=f"sps{ti}_{tq}")
                        nc.tensor.matmul(sps[:], pm[:], sb[:, ts],
                                         start=True, stop=True)
                        nc.vector.tensor_mul(t2[:, ts], sps[:], sc[:, ts])
                    nc.vector.tensor_mul(t1[:], sb[:], cc[:])
                    nc.vector.tensor_add(rot[:], t1[:], t2[:])
                    rots.append(rot)
                qrot, krot = rots

                for tq in range(2):
                    o_ps = [psO.tile([DH + 1, 512], f32, tag="psO",
                                     name=f"ops{hh}") for hh in range(2)]
                    # software pipeline: scores/exp of round r+1 are issued
                    # before the AV matmuls of round r, so the in-order PE
                    # never waits on ACT's exp. Each round packs 2 key-chunks
                    # x 2 heads of scores into one 4-bank tile -> one exp.
                    prev = None
                    for r in range(EC // 2):
                        scps = psS.tile([128, 4, 512], f32, tag="psS",
                                        name="sc4")
                        for j in range(2):
                            tkc = 2 * r + j
                            for hh in range(2):
                                qh = qrot[hh * 64:(hh + 1) * 64, :]
                                kh = krot[hh * 64:(hh + 1) * 64, :]
                                nc.tensor.matmul(
                                    scps[:, 2 * j + hh],
                                    kh[:, tkc * 128:(tkc + 1) * 128],
                                    qh[:, tq * 512:(tq + 1) * 512],
                                    start=True, stop=True)
                        ext = expp.tile([128, 4, 512], bf16, tag="ext")
                        nc.scalar.activation(ext[:], scps[:], FT.Exp,
                                             scale=SCALE)
                        if prev is not None:
                            for j in range(2):
                                tkc = 2 * (r - 1) + j
                                for hh in range(2):
                                    nc.tensor.matmul(
                                        o_ps[hh][:],
                                        vvt[:, tkc, 2 * hp + hh, :],
                                        prev[:, 2 * j + hh],
                                        start=(tkc == 0), stop=False)
                        prev = ext
                    for j in range(2):
                        tkc = EC - 2 + j
                        for hh in range(2):
                            nc.tensor.matmul(
                                o_ps[hh][:],
                                vvt[:, tkc, 2 * hp + hh, :],
                                prev[:, 2 * j + hh],
                                start=False, stop=(tkc == EC - 1))
                    for hh in range(2):
                        rc = smallp.tile([1, 512], f32, tag="rc")
                        nc.vector.reciprocal(rc[:], o_ps[hh][DH:DH + 1, :])
                        rcb = smallp.tile([64, 512], f32, tag="rcb")
                        nc.gpsimd.partition_broadcast(rcb[:], rc[:])
                        nc.vector.tensor_mul(
                            attn_sb[hh * 64:(hh + 1) * 64, hp,
                                    tq * 512:(tq + 1) * 512],
                            o_ps[hh][0:DH, :], rcb[:])

            # ---- out-proj: y[t, o] = attnT.T-chunks @ WoT + bo ----
            for tcn in range(EC):
                for oh in range(2):
                    yps = psP.tile([128, 512], f32, tag="psP",
                                   name=f"yps{oh}")
                    for ec in range(EC):
                        nc.tensor.matmul(
                            yps[:],
                            attn_sb[:, ec, tcn * 128:(tcn + 1) * 128],
                            wo_sb[:, ec, oh * 512:(oh + 1) * 512],
                            start=(ec == 0), stop=(ec == EC - 1))
                    ysb = yp.tile([128, 512], f32, tag="y")
                    nc.vector.tensor_add(ysb[:], yps[:],
                                         bo_sb[:, oh * 512:(oh + 1) * 512])
                    nc.sync.dma_start(
                        y_d.ap()[b, tcn * 128:(tcn + 1) * 128,
                                 oh * 512:(oh + 1) * 512], ysb[:])

    nc.compile()
    return nc


def _host_prep(inputs):
    import ml_dtypes
    bf16 = ml_dtypes.bfloat16

    x = np.asarray(inputs["hidden_states"], dtype=np.float32)
    rope_pos = np.asarray(inputs["rope_pos"])

    # per-head permutation: [h-half evens, w-half evens, h-half odds, w-half odds]
    p64 = np.concatenate([
        np.arange(0, HALF, 2), np.arange(HALF, DH, 2),
        np.arange(1, HALF, 2), np.arange(HALF + 1, DH, 2)])
    perm = np.concatenate([h * DH + p64 for h in range(H)])

    wqt = np.ascontiguousarray(np.asarray(inputs["Wq"], np.float32).T[:, perm]).astype(bf16)
    wkt = np.ascontiguousarray(np.asarray(inputs["Wk"], np.float32).T[:, perm]).astype(bf16)
    wvt = np.ascontiguousarray(np.asarray(inputs["Wv"], np.float32).T).astype(bf16)
    wot = np.ascontiguousarray(np.asarray(inputs["Wo"], np.float32).T).astype(bf16)
    bq_p = np.asarray(inputs["bq"], np.float32)[perm]
    bk_p = np.asarray(inputs["bk"], np.float32)[perm]
    bv = np.asarray(inputs["bv"], np.float32)
    bo = np.asarray(inputs["bo"], np.float32)

    # bqk [128, 2*HP]: col ti*HP+hp = bias for slab hp of (q if ti==0 else k)
    bqk = np.empty((128, 2 * HP), np.float32)
    for hp in range(HP):
        bqk[:, hp] = bq_p[hp * 128:(hp + 1) * 128]
        bqk[:, HP + hp] = bk_p[hp * 128:(hp + 1) * 128]
    bv_rep = np.ascontiguousarray(np.broadcast_to(bv, (128, E)))
    bo_rep = np.ascontiguousarray(np.broadcast_to(bo, (128, E)))

    # trig tables, f32 pipeline mirroring the reference, then bf16
    idx = np.arange(QUARTER, dtype=np.float32)
    inv = (np.float32(THETA) ** (np.float32(-2.0) * idx / np.float32(QUARTER))
           ).astype(np.float32)
    pos = rope_pos.astype(np.float32)                    # [B, T, 2]
    ang_h = pos[:, :, 0:1] * inv                         # [B, T, 16]
    ang_w = pos[:, :, 1:2] * inv
    ch, cw = np.cos(ang_h), np.cos(ang_w)
    sh, sw = np.sin(ang_h), np.sin(ang_w)
    cos64 = np.concatenate([ch, cw, ch, cw], axis=2)     # [B, T, 64]
    sin64 = np.concatenate([-sh, -sw, sh, sw], axis=2)
    ccat = np.ascontiguousarray(np.transpose(cos64, (0, 2, 1)))  # [B, 64, T]
    scat = np.ascontiguousarray(np.transpose(sin64, (0, 2, 1)))
    ccat = np.ascontiguousarray(np.concatenate([ccat, ccat], axis=1)).astype(bf16)
    scat = np.ascontiguousarray(np.concatenate([scat, scat], axis=1)).astype(bf16)

    pmat = np.zeros((128, 128), np.float32)
    for base in (0, 64):
        pmat[base:base + 32, base + 32:base + 64] = np.eye(32)
        pmat[base + 32:base + 64, base:base + 32] = np.eye(32)
    pmat = pmat.astype(bf16)

    xt_all = np.transpose(x, (0, 2, 1)).astype(bf16)     # [B, E, T]

    in_maps = []
    for c in range(N_CORES):
        bs = slice(c * BPC, (c + 1) * BPC)
        in_maps.append({
            "xt": np.ascontiguousarray(xt_all[bs]),
            "wqt": wqt, "wkt": wkt, "wvt": wvt, "wot": wot,
            "pmat": pmat,
            "ccat": np.ascontiguousarray(ccat[bs]),
            "scat": np.ascontiguousarray(scat[bs]),
            "bqk": bqk, "bv": bv_rep, "bo": bo_rep,
        })
    return in_maps


PROFILE = False
LAST_RESULT = None


def kernel(**inputs):
    global _compiled_nc, LAST_RESULT
    from concourse.bass_utils import run_bass_kernel_spmd

    if _compiled_nc is None:
        _compiled_nc = _build_nc()
    in_maps = _host_prep(inputs)
    res = run_bass_kernel_spmd(_compiled_nc, in_maps, list(range(N_CORES)),
                               trace=PROFILE)
    LAST_RESULT = res
    out = np.concatenate([res.results[c]["y"] for c in range(N_CORES)], axis=0)
    return out.astype(np.float32)


# revision 33
# speedup vs baseline: 1.5639x; 1.4289x over previous
"""Trainium2 Bass kernel for CLIPAttention with 2D interleaved RoPE.

Problem: B=16, T=1024, E=1024, H=16, DH=64, f32 in/out.
Sharding: data-parallel over batch across 8 NeuronCores (2 batches/core).

v3: bf16 matmul operands (f32 PSUM accumulation, f32 softmax exp on ACT),
restructured for overlap:
  - dedicated PSUM pools per stage (proj / scores / attn-out) so the Tile
    scheduler can run stage N+1 matmuls during stage N's ACT/DVE work
  - attention loops tq-outer, heads of a pair interleaved (their scores
    matmuls use disjoint 64-row groups, so weight loads overlap matmuls)
  - weights Wv/Wo loaded once for both batches; x/trig DMA'd per e-chunk and
    double-buffered across batches
  - Q/K PSUM->SBUF copies (with bias) on DVE, exp on ACT

Per-core algorithm per batch:
  host prep:  xT [E,T] bf16; W^T [e,o] bf16; q/k output dims permuted per
              head so RoPE pair partners sit 32 partitions apart; trig
              tables ccat/scat [128,T] bf16 with signs folded; P (block-swap)
              bf16; biases f32.
  device:     V = x@Wv^T +bv (natural [t,o]) -> VV tiles [tk,65] bf16 with
              ones col per head; per head-pair hp:
                QT,KT = (W^T slab).T @ xT (+bias via DVE, out bf16)
                rope: rot = q*ccat + (P@q)*scat  (swap via PE matmul)
                per tq-half, heads interleaved:
                  scoresT[tk,tq] = KT.T@QT; expT = exp(scale*scoresT) bf16
                  outT[0:65] = [V|1].T @ expT (accum over tk chunks)
                  denom = row 64; attnT = outT[0:64] * bcast(1/denom) -> bf16
              y = attnT.T-chunks @ Wo^T + bo  (f32, natural [t,o]) -> DRAM
Softmax skips max-subtraction: |scores*scale| <~ 8 for these inputs, exp is
exact in fp32 there, and softmax is shift-invariant.
"""
import numpy as np

B, T, E, H = 16, 1024, 1024, 16
DH = E // H            # 64
THETA = 10000.0
N_CORES = 8
BPC = B // N_CORES     # 2 batches per core
HP = H // 2            # 8 head pairs
EC = E // 128          # 8 e-chunks
HALF, QUARTER = DH // 2, DH // 4   # 32, 16
SCALE = float(DH) ** -0.5

_compiled_nc = None


def _build_nc(reps=1, hoist_dma=False):
    import concourse.bacc as bacc
    import concourse.tile as tile
    from concourse import mybir
    from contextlib import ExitStack, nullcontext

    f32 = mybir.dt.float32
    bf16 = mybir.dt.bfloat16
    FT = mybir.ActivationFunctionType

    nc = bacc.Bacc("TRN2", target_bir_lowering=False)

    xt_d = nc.dram_tensor("xt", [BPC, E, T], bf16, kind="ExternalInput")
    wqt_d = nc.dram_tensor("wqt", [E, E], bf16, kind="ExternalInput")
    wkt_d = nc.dram_tensor("wkt", [E, E], bf16, kind="ExternalInput")
    wvt_d = nc.dram_tensor("wvt", [E, E], bf16, kind="ExternalInput")
    wot_d = nc.dram_tensor("wot", [E, E], bf16, kind="ExternalInput")
    pmat_d = nc.dram_tensor("pmat", [128, 128], bf16, kind="ExternalInput")
    ccat_d = nc.dram_tensor("ccat", [BPC, 128, T], bf16, kind="ExternalInput")
    scat_d = nc.dram_tensor("scat", [BPC, 128, T], bf16, kind="ExternalInput")
    bqk_d = nc.dram_tensor("bqk", [128, 2 * HP], f32, kind="ExternalInput")
    bv_d = nc.dram_tensor("bv", [128, E], f32, kind="ExternalInput")
    bo_d = nc.dram_tensor("bo", [128, E], f32, kind="ExternalInput")
    y_d = nc.dram_tensor("y", [BPC, T, E], f32, kind="ExternalOutput")

    def wslab_ap(w, hp):
        return w.ap().rearrange("(c p) o -> p c o", p=128)[:, :, hp * 128:(hp + 1) * 128]

    with tile.TileContext(nc) as tc, ExitStack() as ctx:
        pre = ctx.enter_context(tc.tile_pool(name="pre", bufs=1)) \
            if hoist_dma else None
        hoisted = {}
        if hoist_dma:
            # stage the big inputs once, outside the timing loop
            for b in range(BPC):
                hx = pre.tile([128, EC, T], bf16, tag=f"hx{b}")
                nc.sync.dma_start(
                    hx[:], xt_d.ap()[b].rearrange("(c p) t -> p c t", p=128))
                hc = pre.tile([128, T], bf16, tag=f"hc{b}")
                nc.sync.dma_start(hc[:], ccat_d.ap()[b])
                hs = pre.tile([128, T], bf16, tag=f"hs{b}")
                nc.sync.dma_start(hs[:], scat_d.ap()[b])
                hoisted[b] = (hx, hc, hs)
            hwv = pre.tile([128, EC, E], bf16, tag="hwv")
            nc.sync.dma_start(
                hwv[:], wvt_d.ap().rearrange("(c p) o -> p c o", p=128))
            hwo = pre.tile([128, EC, E], bf16, tag="hwo")
            nc.sync.dma_start(
                hwo[:], wot_d.ap().rearrange("(c p) o -> p c o", p=128))
        ctx.enter_context(tc.For_i(0, reps, 1) if reps > 1 else nullcontext())
        const = ctx.enter_context(tc.tile_pool(name="const", bufs=1))
        if not hoist_dma:
            wvwo = ctx.enter_context(tc.tile_pool(name="wvwo", bufs=2))
            trig = ctx.enter_context(tc.tile_pool(name="trig", bufs=1))
            xtp = ctx.enter_context(tc.tile_pool(name="xtp", bufs=2))
        wslab = ctx.enter_context(tc.tile_pool(name="wslab", bufs=1))
        qkp = ctx.enter_context(tc.tile_pool(name="qkp", bufs=2))
        rotp = ctx.enter_context(tc.tile_pool(name="rotp", bufs=2))
        tmpp = ctx.enter_context(tc.tile_pool(name="tmpp", bufs=3))
        vvp = ctx.enter_context(tc.tile_pool(name="vvp", bufs=2))
        expp = ctx.enter_context(tc.tile_pool(name="expp", bufs=3))
        attnp = ctx.enter_context(tc.tile_pool(name="attnp", bufs=1))
        smallp = ctx.enter_context(tc.tile_pool(name="smallp", bufs=2))
        yp = ctx.enter_context(tc.tile_pool(name="yp", bufs=2))
        psP = ctx.enter_context(tc.tile_pool(name="psP", bufs=2, space="PSUM"))
        psS = ctx.enter_context(tc.tile_pool(name="psS", bufs=1, space="PSUM"))
        psO = ctx.enter_context(tc.tile_pool(name="psO", bufs=2, space="PSUM"))

        pm = const.tile([128, 128], bf16, tag="pm")
        nc.sync.dma_start(pm[:], pmat_d.ap())
        bqk_sb = const.tile([128, 2 * HP], f32, tag="bqk")
        nc.sync.dma_start(bqk_sb[:], bqk_d.ap())
        bv_sb = const.tile([128, E], f32, tag="bv")
        nc.sync.dma_start(bv_sb[:], bv_d.ap())
        bo_sb = const.tile([128, E], f32, tag="bo")
        nc.sync.dma_start(bo_sb[:], bo_d.ap())

        # Wv / Wo: shared by both batches, load once (chunked per e-chunk).
        # Wo's DMA is queued after batch 0's inputs — it is needed ~300us in.
        if hoist_dma:
            wv_sb, wo_sb = hwv, hwo
        else:
            wv_sb = wvwo.tile([128, EC, E], bf16, tag="wvwo", name="wv")
            wo_sb = wvwo.tile([128, EC, E], bf16, tag="wvwo", name="wo")

        for b in range(BPC):
            if hoist_dma:
                xts, cc, sc = hoisted[b]
            else:
                xts = xtp.tile([128, EC, T], bf16, tag="xts")
                nc.sync.dma_start(
                    xts[:], xt_d.ap()[b].rearrange("(c p) t -> p c t", p=128))
                if b == 0:
                    nc.sync.dma_start(
                        wv_sb[:],
                        wvt_d.ap().rearrange("(c p) o -> p c o", p=128))
                cc = trig.tile([128, T], bf16, tag="cc")
                nc.sync.dma_start(cc[:], ccat_d.ap()[b])
                sc = trig.tile([128, T], bf16, tag="sc")
                nc.sync.dma_start(sc[:], scat_d.ap()[b])
            if b == 0 and not hoist_dma:
                nc.sync.dma_start(
                    wo_sb[:],
                    wot_d.ap().rearrange("(c p) o -> p c o", p=128))

            # ---- V phase: V natural [t, o] for all heads -> VV tiles ----
            vvt = vvp.tile([128, EC, H, DH + 1], bf16, tag="vv")
            nc.gpsimd.memset(vvt[:, :, :, DH:DH + 1], 1.0)
            for tcn in range(EC):
                for oh in range(2):
                    vps = psP.tile([128, 512], f32, tag="psP",
                                   name=f"vps{tcn}_{oh}")
                    for ec in range(EC):
                        nc.tensor.matmul(
                            vps[:],
                            xts[:, ec, tcn * 128:(tcn + 1) * 128],
                            wv_sb[:, ec, oh * 512:(oh + 1) * 512],
                            start=(ec == 0), stop=(ec == EC - 1))
                    nc.vector.tensor_add(
                        vvt[:, tcn, oh * 8:(oh + 1) * 8, 0:DH],
                        vps[:].rearrange("p (h d) -> p h d", d=DH),
                        bv_sb[:, oh * 512:(oh + 1) * 512]
                        .rearrange("p (h d) -> p h d", d=DH))

            attn_sb = attnp.tile([128, EC, T], bf16, tag="attn")

            # Q/K weight slabs for the whole batch: two big DMAs
            wqk_sb = wslab.tile([128, 2, EC, E], bf16, tag="wqk")
            nc.sync.dma_start(
                wqk_sb[:, 0], wqt_d.ap().rearrange("(c p) o -> p c o", p=128))
            nc.sync.dma_start(
                wqk_sb[:, 1], wkt_d.ap().rearrange("(c p) o -> p c o", p=128))

            # ---- per head-pair: Q/K projection + rope + attention ----
            for hp in range(HP):
                rots = []
                for ti in range(2):
                    sb = qkp.tile([128, T], bf16, tag="qksb")
                    rot = rotp.tile([128, T], bf16, tag=("rotq", "rotk")[ti])
                    t1 = tmpp.tile([128, T], bf16, tag="t1")
                    t2 = tmpp.tile([128, T], bf16, tag="t2")
                    for tq in range(2):
                        ts = slice(tq * 512, (tq + 1) * 512)
                        pps = psP.tile([128, 512], f32, tag="psP",
                                       name=f"pps{ti}_{tq}")
                        for ec in range(EC):
                            nc.tensor.matmul(
                                pps[:],
                                wqk_sb[:, ti, ec, hp * 128:(hp + 1) * 128],
                                xts[:, ec, ts],
                                start=(ec == 0), stop=(ec == EC - 1))
                        nc.vector.tensor_scalar_add(
                            sb[:, ts], pps[:],
                            bqk_sb[:, ti * HP + hp:ti * HP + hp + 1])
                        sps = psP.tile([128, 512], f32, tag="psP",
                                       name=f"sps{ti}_{tq}")
                        nc.tensor.matmul(sps[:], pm[:], sb[:, ts],
                                         start=True, stop=True)
                        nc.vector.tensor_mul(t2[:, ts], sps[:], sc[:, ts])
                    nc.vector.tensor_mul(t1[:], sb[:], cc[:])
                    nc.vector.tensor_add(rot[:], t1[:], t2[:])
                    rots.append(rot)
                qrot, krot = rots

                for tq in range(2):
                    o_ps = [psO.tile([DH + 1, 512], f32, tag="psO",
                                     name=f"ops{hh}") for hh in range(2)]
                    # software pipeline: scores/exp of round r+1 are issued
                    # before the AV matmuls of round r, so the in-order PE
                    # never waits on ACT's exp. Each round packs 2 key-chunks
                    # x 2 heads of scores into one 4-bank tile -> one exp.
                    prev = None
                    for r in range(EC // 2):
                        scps = psS.tile([128, 4, 512], f32, tag="psS",
                                        name="sc4")
                        for j in range(2):
                            tkc = 2 * r + j
                            for hh in range(2):
                                qh = qrot[hh * 64:(hh + 1) * 64, :]
                                kh = krot[hh * 64:(hh + 1) * 64, :]
                                nc.tensor.matmul(
                                    scps[:, 2 * j + hh],
                                    kh[:, tkc * 128:(tkc + 1) * 128],
                                    qh[:, tq * 512:(tq + 1) * 512],
                                    start=True, stop=True)
                        ext = expp.tile([128, 4, 512], bf16, tag="ext")
                        nc.scalar.activation(ext[:], scps[:], FT.Exp,
                                             scale=SCALE)
                        if prev is not None:
                            for j in range(2):
                                tkc = 2 * (r - 1) + j
                                for hh in range(2):
                                    nc.tensor.matmul(
                                        o_ps[hh][:],
                                        vvt[:, tkc, 2 * hp + hh, :],
                                        prev[:, 2 * j + hh],
                                        start=(tkc == 0), stop=False)
                        prev = ext
                    for j in range(2):
                        tkc = EC - 2 + j
                        for hh in range(2):
                            nc.tensor.matmul(
                                o_ps[hh][:],
                                vvt[:, tkc, 2 * hp + hh, :],
                                prev[:, 2 * j + hh],
                                start=False, stop=(tkc == EC - 1))
                    for hh in range(2):
                        rc = smallp.tile([1, 512], f32, tag="rc")
                        nc.vector.reciprocal(rc[:], o_ps[hh][DH:DH + 1, :])
                        rcb = smallp.tile([64, 512], f32, tag="rcb")
                        nc.gpsimd.partition_broadcast(rcb[:], rc[:])
                        nc.vector.tensor_mul(
                            attn_sb[hh * 64:(hh + 1) * 64, hp,
                                    tq * 512:(tq + 1) * 512],
                            o_ps[hh][0:DH, :], rcb[:])

            # ---- out-proj: y[t, o] = attnT.T-chunks @ WoT + bo ----
            for tcn in range(EC):
                for oh in range(2):
                    yps = psP.tile([128, 512], f32, tag="psP",
                                   name=f"yps{oh}")
                    for ec in range(EC):
                        nc.tensor.matmul(
                            yps[:],
                            attn_sb[:, ec, tcn * 128:(tcn + 1) * 128],
                            wo_sb[:, ec, oh * 512:(oh + 1) * 512],
                            start=(ec == 0), stop=(ec == EC - 1))
                    ysb = yp.tile([128, 512], f32, tag="y")
                    nc.vector.tensor_add(ysb[:], yps[:],
                                         bo_sb[:, oh * 512:(oh + 1) * 512])
                    nc.sync.dma_start(
                        y_d.ap()[b, tcn * 128:(tcn + 1) * 128,
                                 oh * 512:(oh + 1) * 512], ysb[:])

    nc.compile()
    return nc


def _host_prep(inputs):
    import ml_dtypes
    bf16 = ml_dtypes.bfloat16

    x = np.asarray(inputs["hidden_states"], dtype=np.float32)
    rope_pos = np.asarray(inputs["rope_pos"])

    # per-head permutation: [h-half evens, w-half evens, h-half odds, w-half odds]
    p64 = np.concatenate([
        np.arange(0, HALF, 2), np.arange(HALF, DH, 2),
        np.arange(1, HALF, 2), np.arange(HALF + 1, DH, 2)])
    perm = np.concatenate([h * DH + p64 for h in range(H)])

    wqt = np.ascontiguousarray(np.asarray(inputs["Wq"], np.float32).T[:, perm]).astype(bf16)
    wkt = np.ascontiguousarray(np.asarray(inputs["Wk"], np.float32).T[:, perm]).astype(bf16)
    wvt = np.ascontiguousarray(np.asarray(inputs["Wv"], np.float32).T).astype(bf16)
    wot = np.ascontiguousarray(np.asarray(inputs["Wo"], np.float32).T).astype(bf16)
    bq_p = np.asarray(inputs["bq"], np.float32)[perm]
    bk_p = np.asarray(inputs["bk"], np.float32)[perm]
    bv = np.asarray(inputs["bv"], np.float32)
    bo = np.asarray(inputs["bo"], np.float32)

    # bqk [128, 2*HP]: col ti*HP+hp = bias for slab hp of (q if ti==0 else k)
    bqk = np.empty((128, 2 * HP), np.float32)
    for hp in range(HP):
        bqk[:, hp] = bq_p[hp * 128:(hp + 1) * 128]
        bqk[:, HP + hp] = bk_p[hp * 128:(hp + 1) * 128]
    bv_rep = np.ascontiguousarray(np.broadcast_to(bv, (128, E)))
    bo_rep = np.ascontiguousarray(np.broadcast_to(bo, (128, E)))

    # trig tables, f32 pipeline mirroring the reference, then bf16
    idx = np.arange(QUARTER, dtype=np.float32)
    inv = (np.float32(THETA) ** (np.float32(-2.0) * idx / np.float32(QUARTER))
           ).astype(np.float32)
    pos = rope_pos.astype(np.float32)                    # [B, T, 2]
    ang_h = pos[:, :, 0:1] * inv                         # [B, T, 16]
    ang_w = pos[:, :, 1:2] * inv
    ch, cw = np.cos(ang_h), np.cos(ang_w)
    sh, sw = np.sin(ang_h), np.sin(ang_w)
    cos64 = np.concatenate([ch, cw, ch, cw], axis=2)     # [B, T, 64]
    sin64 = np.concatenate([-sh, -sw, sh, sw], axis=2)
    ccat = np.ascontiguousarray(np.transpose(cos64, (0, 2, 1)))  # [B, 64, T]
    scat = np.ascontiguousarray(np.transpose(sin64, (0, 2, 1)))
    ccat = np.ascontiguousarray(np.concatenate([ccat, ccat], axis=1)).astype(bf16)
    scat = np.ascontiguousarray(np.concatenate([scat, scat], axis=1)).astype(bf16)

    pmat = np.zeros((128, 128), np.float32)
    for base in (0, 64):
        pmat[base:base + 32, base + 32:base + 64] = np.eye(32)
        pmat[base + 32:base + 64, base:base + 32] = np.eye(32)
    pmat = pmat.astype(bf16)

    xt_all = np.transpose(x, (0, 2, 1)).astype(bf16)     # [B, E, T]

    in_maps = []
    for c in range(N_CORES):
        bs = slice(c * BPC, (c + 1) * BPC)
        in_maps.append({
            "xt": np.ascontiguousarray(xt_all[bs]),
            "wqt": wqt, "wkt": wkt, "wvt": wvt, "wot": wot,
            "pmat": pmat,
            "ccat": np.ascontiguousarray(ccat[bs]),
            "scat": np.ascontiguousarray(scat[bs]),
            "bqk": bqk, "bv": bv_rep, "bo": bo_rep,
        })
    return in_maps


PROFILE = False
LAST_RESULT = None


def kernel(**inputs):
    global _compiled_nc, LAST_RESULT
    from concourse.bass_utils import run_bass_kernel_spmd

    if _compiled_nc is None:
        _compiled_nc = _build_nc()
    in_maps = _host_prep(inputs)
    res = run_bass_kernel_spmd(_compiled_nc, in_maps, list(range(N_CORES)),
                               trace=PROFILE)
    LAST_RESULT = res
    out = np.concatenate([res.results[c]["y"] for c in range(N_CORES)], axis=0)
    return out.astype(np.float32)
